# revision 16
# baseline (speedup 1.0000x reference)
"""COOTensorProduct kernel for 8 Trainium2 NeuronCores (bf16 pipeline).

Math: out[b, h] = sum_{i,j} cb[h, i*64+j] * in1[b, i] * in2[b, j]
with in1/in2 [4096, 64], cb [4096, 4096] (a Clebsch-Gordan / Wigner-3j
coupling matrix for irreps '4x0e+4x1o+4x2e+4x3o' x same -> all l3).

cb is 0.1% dense but perfectly block-structured: the 16 (l1,l2) pair
couplings pack block-diagonally into exactly two 128x128 stationary
matrices (49+35+35+9 = 128 and the rest = 128), identical across the
4x4 multiplicity copies (u, v).

Per core (512 batch rows), all in bf16 (tolerance is 2e-2; bf16
end-to-end costs ~5e-3):
  rhs[S][u]  = g1[S,u] (bcast x4) * g2[S,0..3]   (one wide DVE mult,
               [128 part = (pair,m1,m2) rows, 2048 free = 4v x batch])
  psum[S][u] = W_S.T @ rhs slices                (4x 128x128x512 bf16
               matmuls -> one [128,2048] fp32 PSUM tile = 4 banks)
  obuf       = copy psum -> SBUF bf16            (one wide ACT copy)
  DMA out    [128, 2048] bf16 per round, 8 rounds.

vs the fp32 baseline (50.4us): PE 4x faster (1 cyc/col bf16), DMA
traffic halved (6.3MB vs 12.6MB/core), DVE mults at 2x packed mode.

Host does the (static, index-only) gathers/permutes + f32<->bf16 casts;
device does all FLOPs.
"""

import json
import numpy as np
import ml_dtypes

BF16 = ml_dtypes.bfloat16

# ---------------------------------------------------------------- problem
B = 4096
DIM = 64
NCORES = 8
BPC = B // NCORES  # 512 batch rows per core
LMAX = 3
NMULT = 4  # multiplicity of each l in '4x0e+4x1o+4x2e+4x3o'
LS = [l for l in range(LMAX + 1) for _ in range(NMULT)]

# block-diagonal packing of the 16 (l1,l2) pair matrices into 2 stationaries
PAIRS_A = [(3, 3), (3, 2), (2, 3), (1, 1)]
PAIRS_B = [(2, 2), (1, 3), (3, 1), (1, 2), (2, 1), (0, 3), (3, 0),
           (0, 2), (2, 0), (0, 1), (1, 0), (0, 0)]

_decomp_cache = None
_nc_cache = None


def _col_start(l, u):
    return sum((2 * ll + 1) * NMULT for ll in range(l)) + u * (2 * l + 1)


def _build_decomp():
    """Index bookkeeping only (no numerics): which cb entries form the two
    stationary matrices, which in1/in2 columns feed each partition row,
    and which output row h each psum row maps to."""
    global _decomp_cache
    if _decomp_cache is not None:
        return _decomp_cache

    # replicate build_cb_matrix's row layout
    layout = {}
    idx1 = 0
    for l1 in LS:
        idx2 = 0
        for l2 in LS:
            for l3 in range(abs(l1 - l2), l1 + l2 + 1):
                layout.setdefault(l3, []).append((l1, l2, idx1 * DIM + idx2))
            idx2 += 2 * l2 + 1
        idx1 += 2 * l1 + 1
    entry_row = {}
    row = 0
    for l3 in sorted(layout):
        for (l1, l2, co) in sorted(layout[l3], key=lambda x: x[0] * LMAX + x[1]):
            entry_row[(l3, co)] = row
            row += 2 * l3 + 1
    assert row == B

    groups = []
    for pairs in (PAIRS_A, PAIRS_B):
        assert sum((2 * a + 1) * (2 * b + 1) for a, b in pairs) == 128
        c1 = np.zeros((NMULT, 128), dtype=np.int64)
        c2 = np.zeros((NMULT, 128), dtype=np.int64)
        h_of = np.zeros((NMULT, NMULT, 128), dtype=np.int64)
        w_k, w_m, w_h, w_c = [], [], [], []  # W[k,m] = cb[h, c]
        off = 0
        for (l1, l2) in pairs:
            n1, n2 = 2 * l1 + 1, 2 * l2 + 1
            kp = n1 * n2
            kk = np.arange(kp)
            m1, m2 = kk // n2, kk % n2
            for u in range(NMULT):
                c1[u, off:off + kp] = _col_start(l1, u) + m1
            for v in range(NMULT):
                c2[v, off:off + kp] = _col_start(l2, v) + m2
            mm = 0
            for l3 in range(abs(l1 - l2), l1 + l2 + 1):
                n3 = 2 * l3 + 1
                h0 = entry_row[(l3, _col_start(l1, 0) * DIM + _col_start(l2, 0))]
                km, m3m = np.meshgrid(kk, np.arange(n3), indexing="ij")
                w_k.append((off + km).ravel())
                w_m.append((off + mm + m3m).ravel())
                w_h.append((h0 + m3m).ravel())
                w_c.append(((_col_start(l1, 0) + m1[km.ravel()]) * DIM
                            + (_col_start(l2, 0) + m2[km.ravel()])))
                for u in range(NMULT):
                    for v in range(NMULT):
                        h = entry_row[(l3, _col_start(l1, u) * DIM + _col_start(l2, v))]
                        h_of[u, v, off + mm:off + mm + n3] = np.arange(h, h + n3)
                mm += n3
            off += kp
        groups.append({
            "c1": c1, "c2": c2, "h_of": h_of,
            "w_k": np.concatenate(w_k), "w_m": np.concatenate(w_m),
            "w_h": np.concatenate(w_h), "w_c": np.concatenate(w_c),
        })

    # device round r = s*4+u emits columns [r*2048 + v*512 + b] with psum
    # partition p -> output row h_of[s][u, v, p]
    hglob = np.zeros(32 * 128, dtype=np.int64)
    for r in range(8):
        s, u = divmod(r, 4)
        for v in range(NMULT):
            hglob[(r * 4 + v) * 128:(r * 4 + v + 1) * 128] = groups[s]["h_of"][u, v]
    _decomp_cache = (groups, hglob)
    return _decomp_cache


def _split_waits(bir_bytes):
    """This container's walrus build rejects >1 sync-wait per instruction
    ("Too many sync wait commands"). Hoist extra waits onto standalone
    EventSemaphore instructions on the same engine (same lowering raw
    bass wait_ge uses)."""
    bir = json.loads(bir_bytes)
    n = 0
    for fn in bir["functions"]:
        for blk in fn["blocks"]:
            out = []
            for inst in blk["instructions"]:
                si = inst.get("sync_info")
                waits = (si or {}).get("on_wait") or []
                if len(waits) > 1:
                    for w in waits[:-1]:
                        n += 1
                        out.append({
                            "debug": inst.get("debug", 0),
                            "engine": inst["engine"],
                            "ins": [], "outs": [],
                            "name": f"I-wsplit-{n}",
                            "opcode": "EventSemaphore",
                            "sync_info": {"on_update": [], "on_wait": [w]},
                        })
                    si["on_wait"] = [waits[-1]]
                out.append(inst)
            blk["instructions"] = out
    return json.dumps(bir).encode()


def _build_nc():
    """Bass program, identical on all 8 cores (SPMD; per-core data differs).

    8 rounds of: 1 wide DVE mult (bf16 2x) -> 4 bf16 matmuls into one
    [128,2048] 4-bank PSUM tile -> 1 wide ACT copy (fp32 PSUM -> bf16
    SBUF) -> 1 output DMA. PSUM ping-pongs 2x4 banks; copies split
    6 ACT / 2 DVE to balance engine load.
    """
    global _nc_cache
    if _nc_cache is not None:
        return _nc_cache
    import concourse.bass as bass
    import concourse.mybir as mybir
    from concourse.tile import TileContext

    f32 = mybir.dt.float32
    bf16 = mybir.dt.bfloat16
    nc = bass.Bass()
    w = nc.dram_tensor("w", [128, 256], bf16, kind="ExternalInput")
    g = nc.dram_tensor("g", [128, 8192], bf16, kind="ExternalInput")
    o = nc.dram_tensor("o", [128, 16384], bf16, kind="ExternalOutput")

    with TileContext(nc) as tc:
        with (
            tc.tile_pool(name="wpool", bufs=1) as wpool,
            tc.tile_pool(name="gpool", bufs=1) as gpool,
            tc.tile_pool(name="rhspool", bufs=8) as rhspool,
            tc.tile_pool(name="psum", bufs=2, space="PSUM") as psumpool,
            tc.tile_pool(name="opool", bufs=8) as opool,
        ):
            gt = gpool.tile([128, 8192], bf16, name="gt")
            wt = wpool.tile([128, 256], bf16, name="wt")
            # group block s: [g1u0 | g2v0..3 | g1u1 | g1u2 | g1u3]. All
            # input DMAs on ONE ring (FIFO) in consumption order so round r
            # never waits on bytes it doesn't need; output DMAs go on other
            # rings (below) to keep this FIFO input-only
            nc.sync.dma_start(out=gt[:, 0:1536], in_=g[:, 0:1536])
            nc.sync.dma_start(out=gt[:, 1536:2560], in_=g[:, 1536:2560])
            nc.sync.dma_start(out=wt, in_=w[:, :])
            nc.sync.dma_start(out=gt[:, 2560:3584], in_=g[:, 2560:3584])
            nc.sync.dma_start(out=gt[:, 3584:4608], in_=g[:, 3584:4608])
            nc.sync.dma_start(out=gt[:, 4608:6656], in_=g[:, 4608:6656])
            nc.sync.dma_start(out=gt[:, 6656:8192], in_=g[:, 6656:8192])

            # PE warm-up: dummy matmuls spanning the whole input wait keep
            # the HAM clock gate at 2.4GHz for the real rounds (a >3.4us
            # idle gap would drop PE back to 1.2GHz)
            wz = wpool.tile([128, 512], bf16, name="wz")
            nc.vector.memset(wz, 0.0)
            psz = psumpool.tile([128, 2048], f32, tag="ps", name="psz")
            for _ in range(12):
                nc.tensor.matmul(psz[:, 0:512], wz[:, 0:128], wz,
                                 start=True, stop=True)

            def mult(r, rhs=None, half=None):
                s, u = divmod(r, 4)
                go = s * 4096 + (0 if u == 0 else 2048 + u * 512)
                g1u = gt[:, go: go + 512]
                if rhs is None:
                    rhs = rhspool.tile([128, 2048], bf16, tag="rhs",
                                       name="rhs")
                lo, nv = (0, 4) if half is None else (half * 1024, 2)
                g2b = gt[:, s * 4096 + 512 + lo: s * 4096 + 512 + lo + nv * 512]
                nc.vector.tensor_mul(
                    out=rhs[:, lo:lo + nv * 512].rearrange(
                        "p (v b) -> p v b", v=nv),
                    in0=g1u.unsqueeze(1).broadcast_to((128, nv, 512)),
                    in1=g2b.rearrange("p (v b) -> p v b", v=nv))
                return rhs

            # round 0's mult in halves: the first needs only dma chunk 1
            rhs0 = mult(0, half=0)
            mult(0, rhs=rhs0, half=1)
            rhs_q = [rhs0, mult(1), mult(2)]
            for r in range(8):
                s, u = divmod(r, 4)
                if r + 3 < 8:
                    rhs_q.append(mult(r + 3))
                rhs = rhs_q.pop(0)
                ps = psumpool.tile([128, 2048], f32, tag="ps", name="ps")
                for v in range(4):
                    nc.tensor.matmul(
                        ps[:, v * 512:(v + 1) * 512],
                        wt[:, s * 128:(s + 1) * 128],
                        rhs[:, v * 512:(v + 1) * 512],
                        start=True, stop=True)
                ot = opool.tile([128, 2048], bf16, tag="ot", name="ot")
                nc.scalar.copy(out=ot[:, 0:1664], in_=ps[:, 0:1664])
                nc.vector.tensor_copy(out=ot[:, 1664:2048], in_=ps[:, 1664:2048])
                if r == 7:
                    # split the last DMA so each half leaves right after its
                    # own copy - shortens the end-of-kernel flush
                    nc.sync.dma_start(out=o[:, r * 2048:r * 2048 + 1664],
                                      in_=ot[:, 0:1664])
                    nc.sync.dma_start(out=o[:, r * 2048 + 1664:(r + 1) * 2048],
                                      in_=ot[:, 1664:2048])
                elif r >= 5:
                    # input FIFO has drained by now; reuse the sync ring
                    nc.sync.dma_start(out=o[:, r * 2048:(r + 1) * 2048], in_=ot)
                else:
                    # early outputs ride the idle SWDGE ring so they don't
                    # queue behind pending input chunks on the sync FIFO
                    nc.gpsimd.dma_start(out=o[:, r * 2048:(r + 1) * 2048],
                                        in_=ot)

    orig = nc.to_json_bytes
    nc.to_json_bytes = lambda: _split_waits(orig())
    _nc_cache = nc
    return nc


def kernel(in1, in2, cb, _want_stats=False):
    from concourse.bass_utils import run_bass_kernel_spmd

    in1 = np.ascontiguousarray(np.asarray(in1, dtype=np.float32))
    in2 = np.ascontiguousarray(np.asarray(in2, dtype=np.float32))
    cb = np.asarray(cb, dtype=np.float32)
    groups, hglob = _build_decomp()

    # stationaries extracted straight from cb (no wigner math needed)
    wmat = np.zeros((2, 128, 128), dtype=np.float32)
    for s, g in enumerate(groups):
        wmat[s][g["w_k"], g["w_m"]] = cb[g["w_h"], g["w_c"]]
    whost = np.empty((128, 256), dtype=BF16)
    whost[:, 0:128] = wmat[0].astype(BF16)
    whost[:, 128:256] = wmat[1].astype(BF16)

    in_maps = []
    for c in range(NCORES):
        sl = slice(c * BPC, (c + 1) * BPC)
        b1t = in1[sl].T.astype(BF16)
        b2t = in2[sl].T.astype(BF16)
        gh = np.empty((128, 8192), dtype=BF16)
        for s, g in enumerate(groups):
            # block layout: [g1u0 | g2v0..3 | g1u1..3]
            gh[:, s * 4096: s * 4096 + 512] = b1t[g["c1"][0]]
            for v in range(NMULT):
                gh[:, s * 4096 + 512 + v * 512: s * 4096 + 512 + (v + 1) * 512] = \
                    b2t[g["c2"][v]]
            for u in range(1, NMULT):
                gh[:, s * 4096 + 2048 + u * 512: s * 4096 + 2048 + (u + 1) * 512] = \
                    b1t[g["c1"][u]]
        in_maps.append({"w": whost, "g": gh})

    nc = _build_nc()
    import os
    trace = bool(int(os.environ.get("KERNEL_TRACE", "0")))
    res = run_bass_kernel_spmd(nc, in_maps, core_ids=list(range(NCORES)),
                               trace=trace)

    out = np.empty((B, B), dtype=np.float32)
    for c in range(NCORES):
        # [128 p, 8 r, 4 v, 512 b] -> [r, v, p, b] -> [4096 rows, 512 b]
        oc = np.asarray(res.results[c]["o"]).astype(np.float32)
        oc = oc.reshape(128, 8, 4, 512).transpose(1, 2, 0, 3).reshape(4096, 512)
        out[c * BPC:(c + 1) * BPC, hglob] = oc.T
    if _want_stats:
        return out, res
    return out


if __name__ == "__main__":
    rng = np.random.default_rng(0)
    a = rng.standard_normal((B, DIM)).astype(np.float32)
    b = rng.standard_normal((B, DIM)).astype(np.float32)
    cb = np.load("/tmp/cb.npy")
    out = kernel(a, b, cb)
    outer = np.einsum("bi,bj->bij", a, b).reshape(B, -1)
    exp = outer @ cb.T
    print("rel err:", np.linalg.norm(out - exp) / np.linalg.norm(exp))
